# revision 1
# baseline (speedup 1.0000x reference)
"""Trainium2 Bass kernel for nn_PositionalScore.

Math (L=8192, D=64, T=9, P=131072, Q=65536):
  out = sum_t sum_p emb[i_tp] @ W_t @ emb[j_tp]  + P * sum(b)
        + 7 clamped-table-lookup sums over Q indices each.

Strategy (8-way data parallel over pairs / table indices):
  - Pair bilinear term: sum_p e_i W_t e_j = <sum_p e_i (x) e_j, W_t>_F.
    Each core gathers its 2*16384 embedding rows per t via SWDGE dma_gather
    (256B rows), PE accumulates G_t = sum_p outer(e_i, e_j) in PSUM via
    128-pair matmuls (lhsT=Ei [128,64], rhs=Ej [128,64]), then DVE takes the
    Frobenius inner product with W_t.
  - Table terms: DVE builds per-partition histograms of the 8192 local
    indices per table (is_equal per bin, is_ge for the clamp bin) and dots
    them with the table values; the b-term is folded in as a constant
    histogram column.
  - gpsimd partition_all_reduce -> one f32 scalar per core; host sums 8.
"""

import numpy as np

import concourse.bass as bass  # noqa: F401  (registers engine classes)
import concourse.bacc as bacc
from concourse import mybir, bass_isa
from concourse.bass_utils import run_bass_kernel_spmd
from concourse.library_config import mlp

L, D, T, P, Q = 8192, 64, 9, 131072, 65536
N_CORES = 8
PC = P // N_CORES          # pairs per core per t
QC = Q // N_CORES          # table idxs per core per table
BATCH_IDXS = 1024          # gathered rows per dma_gather (HW fails >= 8192)
NB = T * (2 * PC) // BATCH_IDXS   # gather batches per core
IDX_COLS = NB * (BATCH_IDXS // 16)  # 18432 int16 idx columns
CPB = BATCH_IDXS // 16     # idx columns per batch
EBC = BATCH_IDXS // 128    # embedding-buffer columns per batch
MPB = EBC // 2             # matmuls per batch
BPT = NB // T              # batches per t slice

_NC_CACHE = {}


def build_program(reps: int = 1):
    A = mybir.AluOpType
    nc = bacc.Bacc("TRN2", target_bir_lowering=False, debug=False,
                   num_devices=N_CORES, num_swdge_queues=4)
    emb_d = nc.dram_tensor("emb", [L, D], mybir.dt.float32, kind="ExternalInput")
    gidx_d = nc.dram_tensor("gidx", [128, IDX_COLS], mybir.dt.int16,
                            kind="ExternalInput")
    tabidx_d = nc.dram_tensor("tabidx", [128, 512], mybir.dt.int32,
                              kind="ExternalInput")
    wsb_d = nc.dram_tensor("wsb", [64, T * 64], mybir.dt.float32,
                           kind="ExternalInput")
    tabs_d = nc.dram_tensor("tabs", [128, 240], mybir.dt.float32,
                            kind="ExternalInput")
    out_d = nc.dram_tensor("out", [1, 1], mybir.dt.float32,
                           kind="ExternalOutput")

    from contextlib import ExitStack
    with ExitStack() as stack, nc.Block() as block:
        ec = stack.enter_context
        gidx_s = ec(nc.sbuf_tensor("gidx_s", [128, IDX_COLS], mybir.dt.int16))
        eb0 = ec(nc.sbuf_tensor("eb0", [128, EBC, 64], mybir.dt.float32))
        eb1 = ec(nc.sbuf_tensor("eb1", [128, EBC, 64], mybir.dt.float32))
        eb2 = ec(nc.sbuf_tensor("eb2", [128, EBC, 64], mybir.dt.float32))
        tabidx_s = ec(nc.sbuf_tensor("tabidx_s", [128, 512], mybir.dt.int32))
        idxf = ec(nc.sbuf_tensor("idxf", [128, 512], mybir.dt.float32))
        scr = ec(nc.sbuf_tensor("scr", [128, 64], mybir.dt.float32))
        e0c = ec(nc.sbuf_tensor("e0c", [128, 64], mybir.dt.float32))
        comb = ec(nc.sbuf_tensor("comb", [128, 64], mybir.dt.float32))
        cnt = ec(nc.sbuf_tensor("cnt", [128, 240], mybir.dt.float32))
        tabs_s = ec(nc.sbuf_tensor("tabs_s", [128, 240], mybir.dt.float32))
        ttrash = ec(nc.sbuf_tensor("ttrash", [128, 240], mybir.dt.float32))
        wsb_s = ec(nc.sbuf_tensor("wsb_s", [64, T * 64], mybir.dt.float32))
        prod = ec(nc.sbuf_tensor("prod", [64, T * 64], mybir.dt.float32))
        tab_e = ec(nc.sbuf_tensor("tab_e", [128, 1], mybir.dt.float32))
        bil_e = ec(nc.sbuf_tensor("bil_e", [64, 1], mybir.dt.float32))
        red = ec(nc.sbuf_tensor("red", [128, 1], mybir.dt.float32))
        Sa = ec(nc.psum_tensor("Sa", [64, 512], mybir.dt.float32))
        Sb = ec(nc.psum_tensor("Sb", [64, 64], mybir.dt.float32))
        io = ec(nc.semaphore("io"))
        gsems = [ec(nc.semaphore(f"gsem{i}")) for i in range(3)]
        psem = ec(nc.semaphore("psem"))
        dsem = ec(nc.semaphore("dsem"))
        vsem = ec(nc.semaphore("vsem"))
        rsem = ec(nc.semaphore("rsem"))
        ebufs = [eb0, eb1, eb2]

        @block.sync
        def _(sync):
            sync.dma_start(gidx_s[:], gidx_d[:]).then_inc(io, 16)
            sync.dma_start(tabidx_s[:], tabidx_d[:]).then_inc(io, 16)
            sync.dma_start(wsb_s[:], wsb_d[:]).then_inc(io, 16)
            sync.dma_start(tabs_s[:], tabs_d[:]).then_inc(io, 16)
            for r in range(reps):
                sync.wait_ge(rsem, r + 1)
                sync.wait_ge(io, 64 + 16 * r)
                sync.dma_start(out_d[:], red[0:1, :]).then_inc(io, 16)

        @block.gpsimd
        def _(g):
            g.load_library(mlp)
            g.wait_ge(io, 64)
            for r in range(reps):
                for b in range(NB):
                    gb = r * NB + b   # global batch number
                    if gb >= 3:
                        g.wait_ge(psem, gb - 2)
                        # same-sem issuer wait: orders this slot's DMA incs
                        g.wait_ge(gsems[gb % 3], 16 * (gb // 3))
                    g.dma_gather(
                        ebufs[gb % 3][:], emb_d[:],
                        gidx_s[:, b * CPB:(b + 1) * CPB],
                        BATCH_IDXS, BATCH_IDXS, D,
                        queue_num=gb % 4,
                    ).then_inc(gsems[gb % 3], 16)
                g.wait_ge(dsem, r + 1)
                if r > 0:
                    g.wait_ge(io, 64 + 16 * r)  # prior out_d DMA drained
                g.partition_all_reduce(red[:], tab_e[:], 128,
                                       bass_isa.ReduceOp.add).then_inc(rsem, 1)
            g.wait_ge(io, 64 + 16 * reps)

        @block.tensor
        def _(pe):
            for r in range(reps):
                if r > 0:
                    pe.wait_ge(dsem, r)  # DVE done reading PSUM from rep r-1
                for b in range(NB):
                    gb = r * NB + b
                    t, ph = b // BPT, b % BPT
                    pe.wait_ge(gsems[gb % 3], 16 * (gb // 3 + 1))
                    eb = ebufs[gb % 3]
                    out = Sa[:, t * 64:(t + 1) * 64] if t < 8 else Sb[:]
                    for m in range(MPB):
                        inst = pe.matmul(
                            out, eb[:, 2 * m, :], eb[:, 2 * m + 1, :],
                            start=(ph == 0 and m == 0),
                            stop=(ph == BPT - 1 and m == MPB - 1),
                        )
                    inst.then_inc(psem, 1)

        @block.vector
        def _(v):
            # The race model gives no implicit same-engine ordering, so every
            # DVE instruction is chained through vsem.
            nv = [0]

            def V(inst):
                inst.then_inc(vsem, 1)
                nv[0] += 1
                v.wait_ge(vsem, nv[0])
                return inst

            v.wait_ge(io, 64)
            for r in range(reps):
                V(v.tensor_copy(idxf[:], tabidx_s[:]))
                # zero only the padding columns; bin/b columns are overwritten
                for lo, hi in ((31, 32), (63, 64), (95, 96), (112, 128),
                               (157, 160), (191, 192), (217, 224), (233, 240)):
                    V(v.memset(cnt[:, lo:hi], 0.0))
                V(v.memset(cnt[:, 224:224 + T], 128.0))
                segs = [(0, 0, 31), (1, 32, 31), (2, 64, 31),
                        (3, 96, 16), (4, 128, 29), (5, 160, 31)]
                for s, base, nbins in segs:
                    seg = idxf[:, s * 64:(s + 1) * 64]
                    for k in range(nbins - 1):
                        V(v.tensor_scalar(scr[:], seg, float(k), 0.0,
                                          A.is_equal, A.add,
                                          accum_out=cnt[:, base + k:base + k + 1]))
                    V(v.tensor_scalar(scr[:], seg, float(nbins - 1), 0.0,
                                      A.is_ge, A.add,
                                      accum_out=cnt[:, base + nbins - 1:base + nbins]))
                # explicit: comb = min(e0,4)*5 + min(e1,4), bins 0..24
                V(v.tensor_scalar(e0c[:], idxf[:, 384:448], 4.0, 5.0,
                                  A.min, A.mult))
                V(v.tensor_scalar(comb[:], idxf[:, 448:512], 4.0, None, A.min))
                V(v.tensor_tensor(comb[:], comb[:], e0c[:], A.add))
                for k in range(25):
                    V(v.tensor_scalar(scr[:], comb[:], float(k), 0.0,
                                      A.is_equal, A.add,
                                      accum_out=cnt[:, 192 + k:192 + k + 1]))
                if r > 0:
                    v.wait_ge(rsem, r)  # gpsimd done reading tab_e of rep r-1
                V(v.tensor_tensor(ttrash[:], cnt[:], tabs_s[:], A.mult))
                V(v.tensor_scalar(ttrash[:], ttrash[:], 1.0, 0.0,
                                  A.mult, A.add, accum_out=tab_e[:]))
                v.wait_ge(psem, NB * (r + 1))
                V(v.tensor_tensor(prod[:, 0:512], Sa[:], wsb_s[:, 0:512],
                                  A.mult))
                V(v.tensor_tensor(prod[:, 512:576], Sb[:], wsb_s[:, 512:576],
                                  A.mult))
                V(v.tensor_scalar(prod[:], prod[:], 1.0, 0.0,
                                  A.mult, A.add, accum_out=bil_e[:]))
                v.tensor_tensor(tab_e[0:64, :], tab_e[0:64, :], bil_e[:],
                                A.add).then_inc(dsem, 1)
                v.wait_ge(dsem, r + 1)

    nc.compile()
    return nc


def _get_nc(reps: int = 1):
    if reps not in _NC_CACHE:
        _NC_CACHE[reps] = build_program(reps)
    return _NC_CACHE[reps]


def make_in_maps(inputs: dict) -> list[dict]:
    emb = np.ascontiguousarray(np.asarray(inputs["embedding"], np.float32))
    W = np.asarray(inputs["W"], np.float32)
    b = np.asarray(inputs["b"], np.float32)
    pair_idx = np.asarray(inputs["pair_idx"], np.int32)
    explicit = np.asarray(inputs["explicit_idx"], np.int32)

    wsb = np.ascontiguousarray(W.transpose(1, 0, 2).reshape(D, T * D))

    tabs_row = np.zeros(240, np.float32)
    tabs_row[0:31] = np.asarray(inputs["hairpin_length"], np.float32)
    tabs_row[32:63] = np.asarray(inputs["bulge_length"], np.float32)
    tabs_row[64:95] = np.asarray(inputs["internal_length"], np.float32)
    tabs_row[96:112] = np.asarray(inputs["internal_symmetry"], np.float32)
    tabs_row[128:157] = np.asarray(inputs["internal_asymmetry"], np.float32)
    tabs_row[160:191] = np.asarray(inputs["helix_length"], np.float32)
    tabs_row[192:217] = np.asarray(inputs["internal_explicit"],
                                   np.float32).reshape(25)
    tabs_row[224:233] = b
    tabs = np.ascontiguousarray(np.tile(tabs_row[None, :], (128, 1)))

    tab_arrs = [np.asarray(inputs[k], np.int32) for k in
                ("hairpin_idx", "bulge_idx", "internal_len_idx",
                 "symmetry_idx", "asymmetry_idx", "helix_idx")]

    in_maps = []
    for c in range(N_CORES):
        pi = pair_idx[:, c * PC:(c + 1) * PC, :]           # [T, PC, 2]
        flat = pi.reshape(T, PC // 128, 128, 2).transpose(0, 1, 3, 2)
        flat = flat.reshape(-1).astype(np.int16)           # [T*2*PC]
        gidx = np.ascontiguousarray(
            np.tile(flat.reshape(-1, 16).T, (8, 1)))       # [128, IDX_COLS]

        cols = [a[c * QC:(c + 1) * QC].reshape(128, 64) for a in tab_arrs]
        cols.append(explicit[c * QC:(c + 1) * QC, 0].reshape(128, 64))
        cols.append(explicit[c * QC:(c + 1) * QC, 1].reshape(128, 64))
        tabidx = np.ascontiguousarray(np.concatenate(cols, axis=1))

        in_maps.append({"emb": emb, "gidx": gidx, "tabidx": tabidx,
                        "wsb": wsb, "tabs": tabs})
    return in_maps


def run(in_maps, reps: int = 1):
    nc = _get_nc(reps)
    return run_bass_kernel_spmd(nc, in_maps, list(range(N_CORES)))


def kernel(**inputs) -> np.ndarray:
    in_maps = make_in_maps(inputs)
    res = run(in_maps, reps=1)
    total = np.float64(0.0)
    for c in range(N_CORES):
        total += np.float64(res.results[c]["out"].reshape(()))
    return np.array(total, dtype=np.float32)

